# revision 54
# baseline (speedup 1.0000x reference)
"""Causal self-attention (T=2048, C=1024, H=16) on 8 trn2 NeuronCores.

Tensor-parallel over heads: core i computes heads 2i, 2i+1 (q/k/v rows
128i:128i+128 of each 1024-row block of wqkv_w, proj_w columns
128i:128i+128), producing a partial output projection; partials are summed
on the host (the all-reduce of the sharding hint).

Per-core Bass/Tile kernel, bf16 matmuls with fp32 PSUM accumulation.
Key structure (v2):
  B. qkvT[j, t] = wqkv.T @ xT, contraction-tile outer so matmuls chase the
     x DMAs (x tiles spread over 3 DMA queues); q/k psum held as [128,2,CH]
     2-bank tiles so each evacuation is one [128,1024] DVE op. v's PE
     transposes are full 128x128 blocks (both heads at once) feeding
     v_aug[k, 65] (ones column = softmax denominator via the PV matmul).
  D. per 512-col t-chunk, k-tiles in PAIRS: both scores of a pair land in
     one [128,2,CH] 2-bank PSUM tile, one [128,<=1024] exp ACTIVATE per
     pair per head (halves ScalarE instruction overhead; the two heads'
     score matmuls row-tile concurrently on the PE via base_partition 0/64).
     Causal affine_select on gpsimd (diagonal k-tiles only); PV pipelined
     behind the scores; the previous chunk's normalize/proj matmuls spread
     through the pair-loop as PE filler.
     Normalize: 1/sums on the DVE (vector.reciprocal on the [1,1024] sums
     row - no ScalarE Ln/Exp, no act-table switches), partition-broadcast
     via K=1 float32r matmul with ones, one DVE multiply per head writing
     into the shared attn[128, T] tile (head h at partitions 64h:64h+64).
  E. partialT[o, t] = projT.T @ attn: ONE K=128 matmul per 128-col o-tile
     (both heads contracted together), evacuated bf16 and stored as
     contiguous 128KB DMAs on rotating queues.
"""

import sys

if "/opt/trn_rl_repo" not in sys.path:
    sys.path.insert(0, "/opt/trn_rl_repo")

import ml_dtypes
import numpy as np

T = 2048
C = 1024
CH = 512  # t-chunk width (one PSUM bank of fp32)
NT = T // CH  # 4 t-chunks
NK = T // 128  # 16 k-tiles
NCT = C // 128  # 8 contraction tiles
N_CORES = 8
PIPE = 4  # scores->PV pipeline depth in k-tile steps

_CACHE = {}


def _patch_act_tables(bacc_mod, mybir):
    """Make Exp and Ln resolve to the one table set containing both, so the
    kernel needs a single ACT_TABLE_LOAD instead of thrashing between
    exp_and_others and natural_log_exp_and_others (~1.3us per reload)."""
    if getattr(bacc_mod, "_attn_act_patch", False):
        return
    orig = bacc_mod.get_activation_tables
    both = {
        mybir.ActivationFunctionType.Exp,
        mybir.ActivationFunctionType.Ln,
        mybir.ActivationFunctionType.Copy,
        mybir.ActivationFunctionType.Identity,
    }

    def patched(arch):
        tabs = dict(orig(arch))
        return {
            name: (funcs if name == "natural_log_exp_and_others" else funcs - both)
            for name, funcs in tabs.items()
        }

    bacc_mod.get_activation_tables = patched
    bacc_mod._attn_act_patch = True


def _build():
    import concourse.tile as tile
    from concourse import bacc, mybir

    _patch_act_tables(bacc, mybir)

    F32 = mybir.dt.float32
    F32R = mybir.dt.float32r
    BF16 = mybir.dt.bfloat16
    EXP = mybir.ActivationFunctionType.Exp
    LN = mybir.ActivationFunctionType.Ln
    IS_GE = mybir.AluOpType.is_ge

    nc = bacc.Bacc(
        "TRN2",
        target_bir_lowering=False,
        debug=False,
        enable_asserts=False,
        num_devices=N_CORES,
        num_swdge_queues=4,
    )
    # x in chunk-major layout [t-chunk, partition, ct, cols]: chunk 0's
    # columns land first so attention can start while chunks 1-3 stream in
    x4 = nc.dram_tensor("x4", [NT, 128, NCT, CH], BF16, kind="ExternalInput").ap()
    wqkv = nc.dram_tensor("wqkv", [C, 384], BF16, kind="ExternalInput").ap()
    projT = nc.dram_tensor("projT", [128, C], BF16, kind="ExternalInput").ap()
    identb = nc.dram_tensor("identb", [128, 128], BF16, kind="ExternalInput").ap()
    ones_f = nc.dram_tensor("ones_f", [128, 128], F32R, kind="ExternalInput").ap()
    bias = nc.dram_tensor("bias", [128, 3], F32, kind="ExternalInput").ap()
    # output as contiguous [chunk, o-tile, 128, 512] bf16 tiles: each store is
    # one fully-contiguous 128KB DMA
    out = nc.dram_tensor("out", [NT, 8, 128, CH], BF16, kind="ExternalOutput").ap()

    with tile.TileContext(nc) as tc:
        with (
            tc.tile_pool(name="big", bufs=1) as big,
            tc.tile_pool(name="expw", bufs=6) as expw_pool,
            tc.tile_pool(name="attn_tmp", bufs=2) as attn_tmp_pool,
            tc.tile_pool(name="outev", bufs=6) as outev_pool,
            tc.tile_pool(name="ps", bufs=1, space="PSUM") as ps,
        ):
            # ---- resident SBUF tensors -------------------------------------
            x_sb = big.tile([128, NT, NCT, CH], BF16, name="x_sb")
            w_sb = big.tile([128, NCT, 384], BF16, name="w_sb")
            projT_sb = big.tile([128, C], BF16, name="projT_sb")
            qT_sb = big.tile([128, T], BF16, name="qT_sb")
            kT_sb = big.tile([128, T], BF16, name="kT_sb")
            vT_sb = big.tile([128, T], BF16, name="vT_sb")
            v_aug0 = big.tile([128, NK, 65], BF16, name="v_aug0")
            v_aug1 = big.tile([128, NK, 65], BF16, name="v_aug1")
            attn_sb = big.tile([128, T], BF16, name="attn_sb")
            ident_sb = big.tile([128, 128], BF16, name="ident_sb")
            ones_sb = big.tile([128, 128], F32R, name="ones_sb")
            bias_sb = big.tile([128, 3], F32, name="bias_sb")

            # w tiles on sync; x streamed chunk-major on the scalar HWDGE +
            # gpsimd SWDGE queues: chunk 0 as four ct-pair slices (so its q/k
            # matmuls chase the arrivals), chunks 1-3 as partition-halves
            # (consumed much later, by D fillers).
            for ct in range(NCT):
                nc.sync.dma_start(
                    out=w_sb[:, ct, :], in_=wqkv[128 * ct : 128 * ct + 128, :]
                )
            nc.sync.dma_start(out=bias_sb, in_=bias)
            nc.sync.dma_start(out=ident_sb, in_=identb)
            # x0 and x1 as half-partition pairs on the two fast queues (both
            # are needed early - x1's q/k build is the first D filler); x2/x3
            # as thirds with sync joining after the small weight loads.
            for c in (0, 1):
                nc.scalar.dma_start(out=x_sb[0:64, c, :, :], in_=x4[c, 0:64])
                nc.gpsimd.dma_start(out=x_sb[64:128, c, :, :], in_=x4[c, 64:128])
            for c in (2, 3):
                nc.scalar.dma_start(out=x_sb[0:44, c, :, :], in_=x4[c, 0:44])
                nc.gpsimd.dma_start(out=x_sb[44:88, c, :, :], in_=x4[c, 44:88])
                nc.sync.dma_start(out=x_sb[88:128, c, :, :], in_=x4[c, 88:128])
            nc.sync.dma_start(out=ones_sb, in_=ones_f)
            nc.sync.dma_start(out=projT_sb, in_=projT)

            nc.vector.memset(v_aug0[:, :, 64:65], 1.0)
            nc.vector.memset(v_aug1[:, :, 64:65], 1.0)

            # ---- stage B: per-chunk q/k/v projections ----------------------
            # Chunk 0 builds in the prelude, chasing its x arrivals; chunks
            # 1-3 build as PE filler inside the earlier D chunks' pair loops.
            def qk_build(part, c):
                g = ps.tile([128, CH], F32, tag="m", bufs=2, name=f"qkb_{part}_{c}")
                cols = slice(128 * part, 128 * part + 128)
                for ct in range(NCT):
                    nc.tensor.matmul(
                        g,
                        w_sb[:, ct, cols],
                        x_sb[:, c, ct, :],
                        start=(ct == 0),
                        stop=(ct == NCT - 1),
                    )
                dest = qT_sb if part == 0 else kT_sb
                nc.vector.tensor_scalar_add(
                    dest[:, CH * c : CH * c + CH], g, bias_sb[:, part : part + 1]
                )

            # v projection + PE transposes, per chunk; chunks 0,1 up front,
            # the rest emitted as PE filler inside stage D's pair-loops.
            def emit_v_chunk(c):
                v_ps = ps.tile([128, CH], F32, tag="m", bufs=2, name=f"vps_{c}")
                for ct in range(NCT):
                    nc.tensor.matmul(
                        v_ps,
                        w_sb[:, ct, 256:384],
                        x_sb[:, c, ct, :],
                        start=(ct == 0),
                        stop=(ct == NCT - 1),
                    )
                nc.vector.tensor_scalar_add(
                    vT_sb[:, CH * c : CH * c + CH], v_ps, bias_sb[:, 2:3]
                )

            def transposes_for(c):
                # full 128x128 transposes: both heads' v in one shot
                for kt in range(4 * c, 4 * c + 4):
                    tr_ps = ps.tile([128, 128], BF16, tag="m", bufs=2, name=f"tr_{kt}")
                    nc.tensor.transpose(
                        tr_ps, vT_sb[:, 128 * kt : 128 * kt + 128], ident_sb
                    )
                    nc.vector.tensor_copy(v_aug0[:, kt, 0:64], tr_ps[:, 0:64])
                    nc.vector.tensor_copy(v_aug1[:, kt, 0:64], tr_ps[:, 64:128])

            # prelude: dummy matmuls on the (early-arriving) first w tile keep
            # the PE active through the x DMA window, so the HAM clock gate is
            # released (1.2 -> 2.4 GHz) before chunk 0's real matmuls start
            warm0 = ps.tile([128, 384], F32, tag="m", bufs=2, name="warm0")
            for r in range(24):
                nc.tensor.matmul(
                    warm0, w_sb[:, 0, 0:128], w_sb[:, 0, :], start=True, stop=True
                )
            # chunk 0's q/k (one 2-bank pair tile) and v
            qk0 = ps.tile([128, 2, CH], F32, tag="s2", bufs=2, name="qk0")
            v0 = ps.tile([128, CH], F32, tag="m", bufs=2, name="v0ps")
            for ct in range(NCT):
                for part in (0, 1):
                    nc.tensor.matmul(
                        qk0[:, part, :],
                        w_sb[:, ct, 128 * part : 128 * part + 128],
                        x_sb[:, 0, ct, :],
                        start=(ct == 0),
                        stop=(ct == NCT - 1),
                    )
                nc.tensor.matmul(
                    v0,
                    w_sb[:, ct, 256:384],
                    x_sb[:, 0, ct, :],
                    start=(ct == 0),
                    stop=(ct == NCT - 1),
                )
            for part in (0, 1):
                dest = qT_sb if part == 0 else kT_sb
                nc.vector.tensor_scalar_add(
                    dest[:, 0:CH], qk0[:, part, :], bias_sb[:, part : part + 1]
                )
            nc.vector.tensor_scalar_add(vT_sb[:, 0:CH], v0, bias_sb[:, 2:3])
            transposes_for(0)

            # ---- stages D+E per t-chunk ------------------------------------
            # Deferred work from chunk c-1, spread through chunk c's pair-loop:
            # P0: at2 copies + Ln/Exp reciprocal (fills the ScalarE slot that
            # used to be an idle chunk-boundary gap), P2: normalize muls,
            # P>=3: projection tiles.
            pending_fin = None  # (pv_ps dict, chunk)
            pending_norm = None  # (at2, chunk)
            pending_proj = None  # chunk index

            def emit_fin(pv_prev, pc):
                at2 = attn_tmp_pool.tile(
                    [65, 2, CH], F32R, tag="attn_tmp", name=f"at2_{pc}"
                )
                for h in (0, 1):
                    nc.vector.tensor_copy(at2[:, h, :], pv_prev[h])
                # 1/sums = exp(-ln(sums)); Ln+Exp share one act table set
                rrow = at2[64:65, :, :]
                nc.scalar.activation(out=rrow, in_=rrow, func=LN)
                nc.scalar.activation(out=rrow, in_=rrow, func=EXP, scale=-1.0)
                return at2

            def emit_norm(at2, pc):
                tcol = slice(CH * pc, CH * pc + CH)
                for h in (0, 1):
                    rb_ps = ps.tile([64, CH], F32, tag="m", bufs=2, name=f"rb_{h}_{pc}")
                    nc.tensor.matmul(
                        rb_ps,
                        ones_sb[64:65, 0:64],
                        at2[64:65, h, :],
                        start=True,
                        stop=True,
                    )
                    nc.vector.tensor_mul(
                        attn_sb[64 * h : 64 * h + 64, tcol], at2[0:64, h, :], rb_ps
                    )

            store_engs = [nc.sync, nc.gpsimd]

            def emit_proj_tile(pc, m, scalar_evac=False, tag="m", tail=False):
                tcol = slice(CH * pc, CH * pc + CH)
                pr_ps = ps.tile([128, CH], F32, tag=tag, bufs=2, name=f"pr_{m}_{pc}")
                nc.tensor.matmul(
                    pr_ps,
                    projT_sb[:, 128 * m : 128 * m + 128],
                    attn_sb[:, tcol],
                    start=True,
                    stop=True,
                )
                ob = outev_pool.tile([128, CH], BF16, tag="outev", name=f"ob_{m}_{pc}")
                if scalar_evac:
                    nc.scalar.copy(ob, pr_ps)
                else:
                    nc.vector.tensor_copy(ob, pr_ps)
                if tail:
                    # tail batch: drain stores over three queues
                    tail_engs = [nc.sync, nc.gpsimd, nc.scalar]
                    tail_engs[m % 3].dma_start(out=out[pc, m], in_=ob)
                else:
                    store_engs[m % 2].dma_start(out=out[pc, m], in_=ob)

            # fillers[(c, P)] -> list of emit thunks: the v projections and
            # transposes spread through the early chunks' pair loops as PE
            # filler, each slice timed to land before its first consumer.
            fillers = {
                (0, 0): [lambda: qk_build(0, 1)],
                (0, 1): [lambda: qk_build(1, 1), lambda: emit_v_chunk(1)],
                (1, 0): [lambda: transposes_for(1)],
                (1, 1): [lambda: qk_build(0, 2)],
                (1, 2): [lambda: qk_build(1, 2)],
                (1, 3): [lambda: emit_v_chunk(2)],
                (2, 0): [lambda: transposes_for(2)],
                (2, 1): [lambda: qk_build(0, 3)],
                (2, 2): [lambda: qk_build(1, 3)],
                (3, 0): [lambda: emit_v_chunk(3)],
                (3, 1): [lambda: transposes_for(3)],
            }

            # Chunk order (0, 1, 2, 3): the first chunks' slack absorbs the
            # stage-B leftovers, and the final chunk is the ScalarE-heaviest
            # (8 exp pairs), giving the PE the deepest well of parallel work
            # while only ONE chunk's norm+proj+store tail remains at the end.
            # PV entries carry their pv tile and stop flag so residual PVs of
            # chunk c can drain inside chunk c+1's pair loop (behind its first
            # scores) instead of stalling the PE queue at the boundary.
            pending = []
            carry = []

            def emit_pv(step):
                for (pj, ph, pw, plane, plo, pvt, stop) in step:
                    nc.tensor.matmul(
                        pvt[:, plo:CH],
                        (v_aug0 if ph == 0 else v_aug1)[:, pj, :],
                        pw[:, plane, plo:CH],
                        start=(pj == 0),
                        stop=stop,
                    )

            for c in (0, 1, 2, 3):
                npair = 2 * c + 2
                pv_ps = {
                    h: ps.tile([65, CH], F32, tag="pv", bufs=2, name=f"pv_{h}_{c}")
                    for h in (0, 1)
                }
                proj_emitted = 0

                for P in range(npair):
                    for h in (0, 1):
                        hrow = slice(64 * h, 64 * h + 64)
                        s2 = ps.tile(
                            [128, 2, CH], F32, tag="s2", bufs=2, name=f"s_{h}_{c}_{P}"
                        )
                        # pair-level column offset: both planes computed from
                        # slo_pair so the pair exp reads only written PSUM
                        # (the second diagonal plane recomputes 128 masked
                        # cols - 128 cycles, cheaper than a separate ACT)
                        slo_pair = max(0, 128 * (2 * P - 4 * c))
                        for i in (0, 1):
                            j = 2 * P + i
                            nc.tensor.matmul(
                                s2[:, i, slo_pair:CH],
                                kT_sb[hrow, 128 * j : 128 * j + 128],
                                qT_sb[hrow, CH * c + slo_pair : CH * c + CH],
                                start=True,
                                stop=True,
                            )
                        w2 = expw_pool.tile(
                            [128, 2, CH], BF16, tag="expw", name=f"w_{h}_{c}_{P}"
                        )
                        nc.scalar.activation(
                            out=w2[:, :, slo_pair:CH],
                            in_=s2[:, :, slo_pair:CH],
                            func=EXP,
                        )
                        for i in (0, 1):
                            j = 2 * P + i
                            diag = j - 4 * c
                            if diag >= 0:
                                # keep exp(score) where t >= k: within the
                                # kept column range f' = f - 128*diag, so
                                # f' - p >= 0
                                lo = 128 * diag if diag > 0 else 0
                                nc.gpsimd.affine_select(
                                    out=w2[:, i, lo:CH],
                                    in_=w2[:, i, lo:CH],
                                    pattern=[[1, CH - lo]],
                                    compare_op=IS_GE,
                                    fill=0.0,
                                    base=0,
                                    channel_multiplier=-1,
                                )
                            pending.append(
                                (
                                    j,
                                    h,
                                    w2,
                                    i,
                                    max(0, 128 * diag),
                                    pv_ps[h],
                                    j == 4 * c + 3,
                                )
                            )
                    if P == 0:
                        if carry:
                            emit_pv(carry)
                            carry = []
                        if pending_fin is not None:
                            pv_prev, pc = pending_fin
                            pending_norm = (emit_fin(pv_prev, pc), pc)
                            pending_fin = None
                    if P == 1 and pending_norm is not None:
                        emit_norm(*pending_norm)
                        pending_norm = None
                    for thunk in fillers.get((c, P), ()):
                        thunk()
                    while len(pending) > 2 * PIPE:
                        step, pending = pending[:2], pending[2:]
                        emit_pv(step)
                    if pending_proj is not None and P >= 2:
                        target = ((P - 1) * 8) // max(npair - 2, 1)
                        while proj_emitted < min(target, 8):
                            emit_proj_tile(pending_proj, proj_emitted)
                            proj_emitted += 1
                if pending_proj is not None:
                    while proj_emitted < 8:
                        emit_proj_tile(pending_proj, proj_emitted)
                        proj_emitted += 1

                carry = pending
                pending = []
                pending_fin = (pv_ps, c)
                pending_proj = c

            # tail: drain the last chunk's residual PVs, then keep the PE
            # clock warm with throwaway matmuls while the DVE/ScalarE run the
            # serial finalize chain, then norm + projections (evacs split
            # ACT/DVE, 4 proj tiles in flight via tag alternation).
            emit_pv(carry)
            warm_ps = ps.tile([128, CH], F32, tag="m", bufs=2, name="warm_ps")
            for r in range(22):
                nc.tensor.matmul(
                    warm_ps,
                    w_sb[:, r % NCT, 0:128],
                    x_sb[:, 0, r % NCT, :],
                    start=True,
                    stop=True,
                )
            pv_prev, pc = pending_fin
            emit_norm(emit_fin(pv_prev, pc), pc)
            for m in range(8):
                # alternate PSUM tags: 4 proj tiles in flight (the s2 slots
                # are free once the last exp has run), evacs split ACT/DVE
                emit_proj_tile(
                    pending_proj,
                    m,
                    scalar_evac=(m % 2 == 1),
                    tag=("m" if m % 2 == 0 else "s2"),
                    tail=True,
                )

    nc.compile()
    return nc


def _get_nc():
    if "nc" not in _CACHE:
        _CACHE["nc"] = _build()
    return _CACHE["nc"]


def _make_in_maps(x, wqkv_w, wqkv_b, proj_w):
    bf = ml_dtypes.bfloat16
    xT = np.asarray(x, np.float32).T.astype(bf)  # [C, T]
    # chunk-major [t-chunk, partition, ct, cols]
    x4 = np.ascontiguousarray(
        xT.reshape(NCT, 128, NT, CH).transpose(2, 1, 0, 3)
    )
    identb = np.eye(128, dtype=bf)
    ones_f = np.ones((128, 128), np.float32)
    scale = np.float32(1.0 / np.sqrt(C))
    in_maps = []
    for i in range(N_CORES):
        rows = []
        biases = []
        for blk, s in ((0, scale), (1, None), (2, None)):
            sl = slice(blk * C + 128 * i, blk * C + 128 * i + 128)
            w = np.asarray(wqkv_w[sl], np.float32)
            b = np.asarray(wqkv_b[sl], np.float32)
            if s is not None:
                w = w * s
                b = b * s
            rows.append(w)
            biases.append(b)
        W = np.concatenate(rows, axis=0)  # [384, 1024]
        B = np.stack(biases, axis=1)  # [128, 3]
        pT = np.asarray(proj_w[:, 128 * i : 128 * i + 128], np.float32).T  # [128, 1024]
        in_maps.append(
            {
                "x4": x4,
                "wqkv": np.ascontiguousarray(W.T.astype(bf)),
                "projT": np.ascontiguousarray(pT.astype(bf)),
                "identb": identb,
                "ones_f": ones_f,
                "bias": np.ascontiguousarray(B),
            }
        )
    return in_maps


def kernel(x, wqkv_w, wqkv_b, proj_w, proj_b, _trace=False, _tmpdir=None):
    from concourse.bass_utils import run_bass_kernel_spmd

    nc = _get_nc()
    in_maps = _make_in_maps(x, wqkv_w, wqkv_b, proj_w)
    res = run_bass_kernel_spmd(
        nc,
        in_maps,
        core_ids=list(range(N_CORES)),
        trace=_trace,
        tmpdir=_tmpdir,
    )
    acc = np.zeros((NT, 8, 128, CH), np.float64)
    for rmap in res.results:
        acc += rmap["out"].astype(np.float64)
    partialT = acc.transpose(1, 2, 0, 3).reshape(C, T)  # [o, t]
    full = partialT.T + np.asarray(proj_b, np.float64)[None, :]
    if _trace:
        _CACHE["last_result"] = res
    return full.astype(np.float32)
